# revision 118
# baseline (speedup 1.0000x reference)
"""TRN2 Bass kernel for nn_MultiHeadAttention_78056735637728.

8-way data parallel over batch (B=8, one batch element per NeuronCore).
Host side: the padding mask m (shared across batch/heads/queries) is applied
EXACTLY by gathering only the unmasked kv positions — masked positions
contribute exp(logit - 1e9) == 0.0 in fp32 to every softmax, so dropping
them is bit-equivalent; k/v are gathered and zero-padded to a multiple of
128 and a per-position bias of -1e9 kills the pad rows inside the fused
exp on device (pad rows only exist in the last kv tile, which is always
routed to the ScalarE exp).

Device kernel (per core). Three co-designed ideas balance the engines
instead of serializing on ScalarE+PE (176.7us -> 116.1us modeled):

1. softmax exp is split across ScalarE (exact fused exp, bf16 out) and
   DVE (bf16 exp2 bit-trick, one tensor_scalar: i16 = logit*
   (scale*log2e*128) + (127-0.0439)*128, bitcast to bf16 == 2^x with
   mantissa-linear interpolation, error centered at +/-3%; softmax's
   num/den ratio cancels most of it). GPSIMD cannot read PSUM on TRN2,
   so the Pool engine only carries DMA triggers + SBUF-side setup.
   Each logits tile is exp'd as two 512 halves on the two engines, in a
   3-deep PSUM rotation, so the PE's logits stream never waits long.
2. PV is computed with P as the STATIONARY operand ([kv, 128-query
   block] tiles, bf16) and [V_h | 1] as the 33-wide moving operand, so
   each matmul streams only 33 rows and the denominator rides along for
   free: PV drops from 512 to 33 moving rows per (head, kv-tile) and
   the result lands query-major [i, (h,d)] with every normalize write
   at base partition 0. NOTE: matmul start=True zeroes the WHOLE PSUM
   bank, so only the first matmul into a shared accumulator carries it.
3. the normalized output O [i, c] is PE-transposed (bf16, into a bf16
   bitcast view of a single-buffer aux PSUM bank) and the output
   projection contracts K=128 per matmul: 8192 moving rows vs 32768.

Logits stay f32r (projected Q/K) with row-packed K=32 stationaries
(4 head-pairs per PE array load); the projections consume bf16 inputs
and weights, which halves the warmup DMA. bo is added on the host.
Measured on HW: rel err 1.06e-2 vs the 2e-2 gate.
"""
import math
import sys
from contextlib import ExitStack

import numpy as np

for _p in ("/opt/trn_rl_repo", "/root/.axon_site/_ro/trn_rl_repo"):
    import os as _os
    if _os.path.isdir(_p) and _p not in sys.path:
        sys.path.insert(0, _p)
        break

import ml_dtypes  # noqa: E402

import concourse.bass as bass  # noqa: E402
import concourse.tile as tile  # noqa: E402
from concourse import bacc, bass_utils, mybir  # noqa: E402
from concourse._compat import with_exitstack  # noqa: E402

F32 = mybir.dt.float32
F32R = mybir.dt.float32r
BF16 = mybir.dt.bfloat16
I16 = mybir.dt.int16
B = 8
S = 2048
D = 256
H = 8
DH = 32
PDIM = 128
N_CORES = 8

LOG2E = 1.4426950408889634
EXP2_CENTER = -0.0439  # centers the mantissa-interp error at +/-3%
# exp-tile engine shares (ScalarE / DVE), tuned via TimelineSim
TUNE_EXP_FRAC = {"A": 0.578, "D": 0.422}
TUNE_PV_DEPTH = 2   # slots between logits(t) and pv(t)
TUNE_PT_BUFS = 5    # SBUF P-tile ring depth
TUNE_EXP_SPLIT = 2  # 1: whole [128,1024] exp tiles @depth-3; 2: halves @6
TUNE_LT_BUFS = 3    # PSUM rotation depth (x2 banks whole / x1 per half)
TUNE_AUX_ON_ACT = False  # steady-state PSUM drains on ScalarE instead of DVE

IN_NAMES = ["qt", "kt", "vt", "wq", "wk", "wv", "wo",
            "bq", "bk", "bo", "bvb", "bvbl", "kvb", "ident", "vone8"]


@with_exitstack
def _mha_kernel(ctx: ExitStack, tc: tile.TileContext, outs, ins, SKV, S=S):
    nc = tc.nc
    (i_qt, i_kt, i_vt, i_wq, i_wk, i_wv, i_wo,
     i_bq, i_bk, i_bo, i_bvb, i_bvbl, i_kvb, i_ident, i_vone8) = ins
    o_ot = outs[0]

    NJ = SKV // PDIM          # kv tiles of 128
    NQC = S // 512            # query chunks of 512
    assert S % 512 == 0 and SKV % PDIM == 0
    scale = 1.0 / math.sqrt(DH)
    # exp2 bit-trick constants (bf16 target): 2^x ~= bitcast_bf16(
    #   round(x*128 + (127+c)*128)) with x = logit*scale*log2e
    ts_mul = scale * LOG2E * 128.0
    ts_add = (127.0 + EXP2_CENTER) * 128.0

    # Each (hp, j) logits tile is exp'd WHOLE on ScalarE (exact exp) or
    # DVE (bf16 exp2 bit-trick) — GPSIMD cannot read PSUM on TRN2, so the
    # Pool engine only carries SBUF-side work and DMA triggers. Whole
    # tiles amortize per-instruction overhead (~1038ns vs 2x655 on ACT).
    # Pads in the last kv tile are harmless for the bit-trick (ones-col/V
    # zeroed there), so every tile is freely assignable. Weighted
    # round-robin keeps each engine's shares evenly spaced.
    EXP_FRAC = dict(TUNE_EXP_FRAC)
    _credits = {k: 0.0 for k in EXP_FRAC}
    _exp_assign = []
    for _ in range(TUNE_EXP_SPLIT * 16 * NJ):
        for k in EXP_FRAC:
            _credits[k] += EXP_FRAC[k]
        pick = max(_credits, key=lambda k: _credits[k])
        _credits[pick] -= 1.0
        _exp_assign.append(pick)

    def exp_engine(g, j, hh=0):
        return _exp_assign[(g * NJ + j) * TUNE_EXP_SPLIT + hh]

    def warm_tag(i):
        # warmup scratch rides the (then-empty) logits rotation tags
        return "lt" if TUNE_EXP_SPLIT == 1 else f"lh{i % 2}"

    def chunks(total, step=512):
        out = []
        c = 0
        while c < total:
            w = min(step, total - c)
            out.append((c, w))
            c += w
        return out

    consts = ctx.enter_context(tc.tile_pool(name="consts", bufs=1))

    # ---- load weights / biases / inputs ----
    # bulk loads ride SP (HWDGE) + vector queue; small weights ride the Pool
    # queue during warmup only (Pool engine is busy with exps later)
    def load2(ap_dram, cols, eng=None, step=512, dt=BF16):
        eng = eng or nc.sync
        ts = []
        for b in range(2):
            t = consts.tile([PDIM, cols], dt, name=f"{ap_dram.name}_sb{b}")
            for c0, w in chunks(cols, step):
                eng.dma_start(t[:, c0:c0 + w],
                              ap_dram[b * PDIM:(b + 1) * PDIM, c0:c0 + w])
            ts.append(t)
        return ts

    def load_bias(ap_dram, name, eng=None):
        t = consts.tile([PDIM, 2], F32, name=name)
        (eng or nc.sync).dma_start(
            t[:], ap_dram.rearrange("(b p) -> p b", p=PDIM))
        return t

    # All loads ride the two HWDGE queues (SP + scalar) so the Pool engine
    # stays free for exp work; K-path (kt chunk 0, wk, bk) leads both
    # queues so the first projection matmul starts ~3.5us in
    acts = ctx.enter_context(tc.tile_pool(name="acts", bufs=1))
    QT = [acts.tile([PDIM, S], F32R, name=f"QT{b}") for b in range(2)]
    KT = [acts.tile([PDIM, SKV], F32R, name=f"KT{b}") for b in range(2)]
    # V interleaved per head with a ones column: cols [h*33, h*33+33) =
    # [V_h (32) | 1]; bf16 (moving operand of the PV matmuls)
    VNE = [acts.tile([PDIM, H * 33], BF16, name=f"VNE{j}") for j in range(NJ)]
    # O^T feature-major bf16, two 128-row blocks (moving side of out-proj)
    OTb = [acts.tile([PDIM, S], BF16, name=f"OTb{cb}") for cb in range(2)]
    wo_sb = [acts.tile([PDIM, D], BF16, name=f"wo_sb{cb}") for cb in range(2)]
    ident = acts.tile([PDIM, PDIM], BF16, name="ident")

    proj_in = ctx.enter_context(tc.tile_pool(name="proj_in", bufs=1))

    def load_chunk(t_list, ap_dram, c0, w, eng1=None):
        for b, eng in ((0, nc.sync), (1, eng1 or nc.scalar)):
            eng.dma_start(t_list[b][:, c0:c0 + w],
                          ap_dram[b * PDIM:(b + 1) * PDIM, c0:c0 + w])

    # three load queues, need-ordered: SP + scalar carry the big activation
    # chunks; the Pool SWDGE queue carries small weights early (it is idle
    # until the first exps at ~7us)
    kt_sb = [proj_in.tile([PDIM, SKV], BF16, name=f"kt_sb{b}")
             for b in range(2)]
    qt_sb = [proj_in.tile([PDIM, S], BF16, name=f"qt_sb{b}")
             for b in range(2)]
    vt_sb = [proj_in.tile([PDIM, SKV], BF16, name=f"vt_sb{b}")
             for b in range(2)]
    # kt block 1 rides the (exp-free) Pool SWDGE queue so the K path loads
    # three-way in parallel
    load_chunk(kt_sb, i_kt, 0, min(512, SKV))        # K-path first
    wk_sb = load2(i_wk, D, eng=nc.gpsimd)
    bk_sb = load_bias(i_bk, "bk_sb", eng=nc.gpsimd)
    # dummy activation so the exp table load happens off the critical path
    warm = consts.tile([PDIM, 1], F32, name="warm")
    nc.scalar.activation(warm[:], bk_sb[:, 0:1],
                         mybir.ActivationFunctionType.Exp)
    load_chunk(qt_sb, i_qt, 0, 512)                   # Q chunk 0 next
    for c0, w in chunks(SKV)[1:]:
        load_chunk(kt_sb, i_kt, c0, w)
    if SKV > 0:
        load_chunk(vt_sb, i_vt, 0, min(512, SKV))     # V chunk 0
    wq_sb = load2(i_wq, D)
    bq_sb = load_bias(i_bq, "bq_sb")
    kvb_sb = consts.tile([PDIM, NJ], F32)
    nc.scalar.dma_start(kvb_sb[:], i_kvb.rearrange("(j p) -> p j", p=PDIM))
    wv_sb = load2(i_wv, D, eng=nc.gpsimd)
    bvb_sb = consts.tile([PDIM, D], F32)
    nc.gpsimd.dma_start(bvb_sb[:], i_bvb[:])
    bvbl_sb = consts.tile([PDIM, D], F32)
    nc.gpsimd.dma_start(bvbl_sb[:], i_bvbl[:])
    for c0, w in chunks(SKV)[1:]:
        load_chunk(vt_sb, i_vt, c0, w, eng1=nc.gpsimd)  # rest of V
    for c0, w in chunks(S)[1:]:
        load_chunk(qt_sb, i_qt, c0, w)
    for cb in range(2):
        nc.sync.dma_start(wo_sb[cb][:], i_wo[cb * PDIM:(cb + 1) * PDIM, :])
    nc.sync.dma_start(ident[:], i_ident[:])
    vone8 = consts.tile([PDIM, 8], BF16, name="vone8")
    nc.scalar.dma_start(vone8[:], i_vone8[:])
    # ones columns of VNE (h*33+32): 1.0 except the pad rows of the last
    # kv tile, which carry 0 so bit-trick pad garbage never reaches num/den
    for j in range(NJ):
        v3 = VNE[j][:].rearrange("p (h c) -> p h c", c=33)
        if j < NJ - 1:
            nc.gpsimd.memset(v3[:, :, 32:33], 1.0)
        else:
            nc.gpsimd.tensor_copy(
                v3[:, :, 32:33],
                vone8[:].rearrange("p (h c) -> p h c", c=1))

    def proj_qk(dst, w_sb, b_sb, x_sb, c0, w, pool, on_act=False,
                warm=False):
        # bias add on ScalarE during warmup (it is idle then) or when DVE
        # is the hotter engine. Steady-state scratch sits in the dedicated
        # single-buffer aux bank, warmup rides the logits rotation.
        for ob in range(2):
            if warm:
                ps = pool.tile([PDIM, 512], F32, tag=warm_tag(ob), name="ps")
            else:
                ps = pool.tile([PDIM, 512], F32, tag="aux", name="ps",
                               bufs=1)
            for ib in range(2):
                nc.tensor.matmul(
                    ps[:, 0:w],
                    lhsT=w_sb[ib][:, ob * PDIM:(ob + 1) * PDIM],
                    rhs=x_sb[ib][:, c0:c0 + w],
                    start=(ib == 0), stop=(ib == 1),
                )
            if on_act:
                nc.scalar.activation(
                    dst[ob][:, c0:c0 + w], ps[:, 0:w],
                    mybir.ActivationFunctionType.Identity,
                    bias=b_sb[:, ob:ob + 1])
            else:
                nc.vector.tensor_scalar_add(
                    dst[ob][:, c0:c0 + w], ps[:, 0:w], b_sb[:, ob:ob + 1])

    def proj_v(j, pool):
        ps = pool.tile([PDIM, D], F32, tag="aux", name="vps", bufs=1)
        for ib in range(2):
            nc.tensor.matmul(
                ps[:],
                lhsT=vt_sb[ib][:, j * PDIM:(j + 1) * PDIM],
                rhs=wv_sb[ib][:],
                start=(ib == 0), stop=(ib == 1),
            )
        v3 = VNE[j][:].rearrange("p (h c) -> p h c", c=33)
        bsrc = bvbl_sb if j == NJ - 1 else bvb_sb
        nc.vector.tensor_add(
            v3[:, :, 0:32],
            ps[:].rearrange("p (h c) -> p h c", c=DH),
            bsrc[:].rearrange("p (h c) -> p h c", c=DH))

    # ---- attention + output projection ----
    # ONE 3-deep PSUM rotation (tag "lt", 2 banks/buf) carries logits AND
    # every projection/transpose scratch: 3-deep gives each tile ~2 slots
    # of reader slack before its banks are rewritten, which rides out the
    # whole-tile exp latency. pv accumulators get the last 2 banks.
    with tc.tile_pool(name="lps", bufs=TUNE_LT_BUFS, space="PSUM") as lps_pool, \
         tc.tile_pool(name="pvps", bufs=2, space="PSUM") as pv_pool, \
         tc.tile_pool(name="psb", bufs=TUNE_PT_BUFS) as p_pool, \
         tc.tile_pool(name="osb", bufs=2) as o_pool, \
         tc.tile_pool(name="outsb", bufs=2) as out_sb, \
         tc.tile_pool(name="norm", bufs=2) as norm_pool:
        pj_pool = pv_pool

        o_tiles = {}  # ic -> one [128, 1024] bf16 tile, cols ib*256 + h*32

        def get_o(ic):
            if ic not in o_tiles:
                o_tiles[ic] = o_pool.tile([PDIM, 1024], BF16, tag="o",
                                          name="o")
            return o_tiles[ic]

        def _logits_mm(out_ap, h, j, i0):
            t = h // 4
            bp = 32 * (h % 4)
            nc.tensor.matmul(
                out_ap,
                lhsT=KT[t][bp:bp + 32, j * PDIM:(j + 1) * PDIM],
                rhs=QT[t][bp:bp + 32, i0:i0 + 512],
                start=True, stop=True,
                tile_position=(bp, 0),
            )

        def _exp_op(kind, dst_ap, src_ap, j):
            if kind == "A":
                nc.scalar.activation(
                    dst_ap, src_ap,
                    mybir.ActivationFunctionType.Exp,
                    bias=kvb_sb[:, j:j + 1], scale=scale)
            else:
                nc.vector.tensor_scalar(
                    dst_ap.bitcast(I16), src_ap, ts_mul, ts_add,
                    op0=mybir.AluOpType.mult, op1=mybir.AluOpType.add)

        def emit_lt(ic, hp, j):
            i0 = ic * 512
            h0 = 2 * hp
            if TUNE_EXP_SPLIT == 1:
                lt = lps_pool.tile([PDIM, 1024], F32, tag="lt", name="lt")
                for hh in range(2):
                    _logits_mm(lt[:, hh * 512:(hh + 1) * 512], h0 + hh, j, i0)
                return [lt]
            lts = []
            for hh in range(2):
                lt = lps_pool.tile([PDIM, 512], F32, tag=f"lh{hh}",
                                   name=f"lh{hh}")
                _logits_mm(lt[:], h0 + hh, j, i0)
                lts.append(lt)
            return lts

        def emit_exp(g, j, lts, pt):
            if TUNE_EXP_SPLIT == 1:
                _exp_op(exp_engine(g, j), pt[:], lts[0][:], j)
            else:
                for hh in range(2):
                    _exp_op(exp_engine(g, j, hh),
                            pt[:, hh * 512:(hh + 1) * 512], lts[hh][:], j)

        def emit_pv(pvt, hp, j, pt):
            h0 = 2 * hp
            for hh in range(2):
                for ib in range(4):
                    nc.tensor.matmul(
                        pvt[:, (hh * 4 + ib) * 33:(hh * 4 + ib + 1) * 33],
                        lhsT=pt[:, hh * 512 + ib * PDIM:
                                hh * 512 + (ib + 1) * PDIM],
                        rhs=VNE[j][:, (h0 + hh) * 33:(h0 + hh + 1) * 33],
                        # start=True zeroes the WHOLE PSUM bank: only the
                        # very first matmul into this accumulator carries it
                        start=(j == 0 and hh == 0 and ib == 0),
                        stop=(j == NJ - 1),
                        tile_position=(0, 0),
                        skip_group_check=True,
                    )

        def emit_norm(pvt, ic, hp):
            # pvt groups g = hh*4+ib: cols [g*33, g*33+32) = num, g*33+32 = den
            ot = get_o(ic)
            pv3 = pvt[:].rearrange("p (g c) -> p g c", c=33)
            rec = norm_pool.tile([PDIM, 8], F32, tag="rec")
            rec3 = rec[:].rearrange("p (g c) -> p g c", c=1)
            nc.vector.reciprocal_approx_fast(rec3[:], pv3[:, :, 32:33])
            # one batched multiply for all 8 (hh, ib) groups of this head
            # pair: out[p, ib, hh, c] = num[p, hh, ib, c] * rec[p, hh*4+ib]
            out4 = ot[:].rearrange("p (ib h c) -> p ib h c", ib=4, c=DH)[
                :, :, 2 * hp:2 * hp + 2, :]
            num4 = pvt[:].rearrange("p (hh ib c) -> p hh ib c", hh=2, c=33)[
                :, :, :, 0:32].transpose([0, 2, 1, 3])
            rec4 = rec[:].rearrange("p (hh ib) -> p hh ib", hh=2).transpose(
                [0, 2, 1]).unsqueeze(3).broadcast_to([PDIM, 4, 2, DH])
            nc.vector.tensor_mul(out4, num4, rec4)

        def transposes(ic, cb):
            ot = get_o(ic)
            tp = pj_pool.tile([PDIM, 512], F32, tag="aux", name="tp",
                              bufs=1)
            tpb = tp[:, 0:256].bitcast(BF16)
            for ib in range(4):
                # start only on the first transpose: start=True zeroes the
                # whole bank and would erase the earlier ib blocks
                nc.tensor.matmul(
                    tpb[:, ib * PDIM:(ib + 1) * PDIM],
                    lhsT=ot[:, ib * 256 + cb * PDIM:
                            ib * 256 + (cb + 1) * PDIM],
                    rhs=ident[:],
                    is_transpose=True,
                    start=(ib == 0), stop=(ib == 3),
                    skip_group_check=True,
                )
            if TUNE_AUX_ON_ACT:
                nc.scalar.activation(
                    OTb[cb][:, ic * 512:(ic + 1) * 512], tpb[:],
                    mybir.ActivationFunctionType.Copy)
            else:
                nc.vector.tensor_copy(
                    OTb[cb][:, ic * 512:(ic + 1) * 512], tpb[:])
            if cb == 1:
                del o_tiles[ic]

        def outproj(ic, ob):
            # bo is added on the host; the PSUM->SBUF drain alternates
            # between ScalarE (as a Copy activation) and DVE for balance
            i0 = ic * 512
            ps = pj_pool.tile([PDIM, 512], F32, tag="aux", name="ops",
                              bufs=1)
            for cb in range(2):
                nc.tensor.matmul(
                    ps[:],
                    lhsT=wo_sb[cb][:, ob * PDIM:(ob + 1) * PDIM],
                    rhs=OTb[cb][:, i0:i0 + 512],
                    start=(cb == 0), stop=(cb == 1),
                )
            ft = out_sb.tile([PDIM, 512], F32, tag="ft")
            if TUNE_AUX_ON_ACT:
                nc.scalar.activation(ft[:], ps[:],
                                     mybir.ActivationFunctionType.Copy)
            else:
                nc.vector.tensor_copy(ft[:], ps[:])
            nc.sync.dma_start(
                o_ot[ob * PDIM:(ob + 1) * PDIM, i0:i0 + 512], ft[:])

        # Q/K projections through the (currently idle) logits PSUM tags so
        # the warmup chain double-buffers; Q chunk 0 last
        for c0, w in chunks(SKV):
            proj_qk(KT, wk_sb, bk_sb, kt_sb, c0, w, lps_pool, on_act=True,
                    warm=True)
        proj_qk(QT, wq_sb, bq_sb, qt_sb, 0, 512, lps_pool, on_act=True,
                warm=True)

        # depth-2 pipelined attention: logits(t) -> exp(t) -> PV(t-2)
        slots = [(ic, hp, j)
                 for ic in range(NQC) for hp in range(4) for j in range(NJ)]
        NT = len(slots)
        pts = {}   # t -> pt tile awaiting PV
        pvts = {}  # (ic, hp) -> pv psum tile

        def do_pv(t2):
            ic2, hp2, j2 = slots[t2]
            if (ic2, hp2) not in pvts:
                pvts[(ic2, hp2)] = pv_pool.tile(
                    [PDIM, 8 * 33], F32, tag="pv", name="pv", bufs=1)
            pvt = pvts[(ic2, hp2)]
            emit_pv(pvt, hp2, j2, pts.pop(t2))
            if j2 == NJ - 1:
                emit_norm(pvt, ic2, hp2)
                del pvts[(ic2, hp2)]

        next_pv = 0

        def pv_depth(t2):
            # group 0 waits an extra slot for the V projections; each
            # group's first PV waits one extra so the previous group's
            # normalize can drain (pv accumulators are single-buffered)
            if t2 < NJ or t2 % NJ == 0:
                return TUNE_PV_DEPTH + 1
            return TUNE_PV_DEPTH

        for t, (ic, hp, j) in enumerate(slots):
            g = ic * 4 + hp
            lts = emit_lt(ic, hp, j)
            # pv (and its normalize, whose DVE ops must precede this
            # slot's exp in the DVE queue) comes right after logits
            while next_pv <= t - pv_depth(next_pv):
                do_pv(next_pv)
                next_pv += 1
            pt = p_pool.tile([PDIM, 1024], BF16, tag="pt")
            emit_exp(g, j, lts, pt)
            pts[t] = pt
            # spread the per-ic extra PE work thinly across slots so the
            # exp engines never starve behind a bunched PE cluster
            if ic == 0 and hp == 0:
                proj_v(j, pj_pool)
            elif hp == 1 and ic + 1 < NQC and j in (0, 2):
                proj_qk(QT, wq_sb, bq_sb, qt_sb,
                        (ic + 1) * 512 + (j // 2) * 256, 256, pj_pool,
                        on_act=TUNE_AUX_ON_ACT)
            elif hp == 2 and ic > 0 and j in (0, 1):
                transposes(ic - 1, j)
            elif hp == 2 and ic > 0 and j in (2, 3):
                outproj(ic - 1, j - 2)
        while next_pv < NT:
            do_pv(next_pv)
            next_pv += 1
        for cb in range(2):
            transposes(NQC - 1, cb)
        for ob in range(2):
            outproj(NQC - 1, ob)


_PROGRAM_CACHE = {}

# DRAM dtypes: f32r matmul operands (same 4-byte payload), bf16 for the
# out-projection weight + transpose identity + pad ones, f32 for the rest
_F32R_INPUTS = set()
_BF16_INPUTS = {"qt", "kt", "vt", "wq", "wk", "wv", "wo", "ident", "vone8"}


def _make_program(SKV, S=S):
    nc = bacc.Bacc("TRN2", target_bir_lowering=False, debug=False,
                   enable_asserts=False, num_devices=1)
    shapes = dict(qt=(D, S), kt=(D, SKV), vt=(D, SKV), wq=(D, D), wk=(D, D),
                  wv=(D, D), wo=(D, D), bq=(D,), bk=(D,), bo=(D,),
                  bvb=(PDIM, D), bvbl=(PDIM, D), kvb=(SKV,),
                  ident=(PDIM, PDIM), vone8=(PDIM, 8))
    def dt_of(k):
        if k in _F32R_INPUTS:
            return F32R
        if k in _BF16_INPUTS:
            return BF16
        return F32
    in_aps = [nc.dram_tensor(k, shapes[k], dt_of(k),
                             kind="ExternalInput").ap()
              for k in IN_NAMES]
    out_ap = nc.dram_tensor("ot", (D, S), F32, kind="ExternalOutput").ap()
    with tile.TileContext(nc) as tc:
        _mha_kernel(tc, [out_ap], in_aps, SKV=SKV, S=S)
    nc.compile()
    return nc


def _get_program(SKV):
    if SKV not in _PROGRAM_CACHE:
        _PROGRAM_CACHE[SKV] = _make_program(SKV)
    return _PROGRAM_CACHE[SKV]


def _prepare_in_maps(q, k, v, m, wq, bq, wk, bk, wv, bv, wo, bo):
    mask = np.asarray(m, np.float32).reshape(-1)
    keep = np.flatnonzero(mask == 0.0)
    skv = len(keep)
    assert skv > 0, "all kv positions masked"
    SKV = max(PDIM, ((skv + PDIM - 1) // PDIM) * PDIM)

    kvb = np.zeros(SKV, np.float32)
    kvb[skv:] = -1e9
    bvb = np.ascontiguousarray(np.tile(np.asarray(bv, np.float32)[None, :],
                                       (PDIM, 1)))
    # last-kv-tile variants with the pad rows zeroed, so bit-trick pad
    # garbage is multiplied by exact zeros in num and den
    valid_last = skv - (SKV // PDIM - 1) * PDIM
    bvbl = bvb.copy()
    bvbl[valid_last:, :] = 0.0
    vone8 = np.ones((PDIM, 8), ml_dtypes.bfloat16)
    vone8[valid_last:, :] = 0.0
    common = dict(
        wq=np.ascontiguousarray(wq, np.float32).astype(ml_dtypes.bfloat16),
        wk=np.ascontiguousarray(wk, np.float32).astype(ml_dtypes.bfloat16),
        wv=np.ascontiguousarray(wv, np.float32).astype(ml_dtypes.bfloat16),
        wo=np.ascontiguousarray(np.asarray(wo, np.float32)).astype(
            ml_dtypes.bfloat16),
        bq=np.ascontiguousarray(bq, np.float32),
        bk=np.ascontiguousarray(bk, np.float32),
        bo=np.ascontiguousarray(bo, np.float32),
        bvb=bvb, bvbl=bvbl, kvb=kvb, vone8=vone8,
        ident=np.eye(PDIM, dtype=ml_dtypes.bfloat16),
    )
    in_maps = []
    for b in range(B):
        kg = np.zeros((D, SKV), ml_dtypes.bfloat16)
        vg = np.zeros((D, SKV), ml_dtypes.bfloat16)
        kg[:, :skv] = np.asarray(k[b], np.float32).T[:, keep].astype(
            ml_dtypes.bfloat16)
        vg[:, :skv] = np.asarray(v[b], np.float32).T[:, keep].astype(
            ml_dtypes.bfloat16)
        in_maps.append(dict(
            qt=np.ascontiguousarray(np.asarray(q[b], np.float32).T).astype(
                ml_dtypes.bfloat16),
            kt=kg, vt=vg, **common))
    return in_maps, SKV


def _run(q, k, v, m, wq, bq, wk, bk, wv, bv, wo, bo, trace=False):
    in_maps, SKV = _prepare_in_maps(q, k, v, m, wq, bq, wk, bk, wv, bv, wo, bo)
    nc = _get_program(SKV)
    last_err = None
    for attempt in range(3):
        try:
            res = bass_utils.run_bass_kernel_spmd(
                nc, in_maps, core_ids=list(range(N_CORES)), trace=trace)
            break
        except Exception as e:  # transient device-unrecoverable states heal
            last_err = e        # on the next NEFF load; retry
    else:
        raise last_err
    bo_f = np.asarray(bo, np.float32)
    out = np.stack([res.results[b]["ot"].T + bo_f for b in range(B)], axis=0)
    return np.ascontiguousarray(out, np.float32), res


def kernel(q, k, v, m, wq, bq, wk, bk, wv, bv, wo, bo):
    out, _ = _run(q, k, v, m, wq, bq, wk, bk, wv, bv, wo, bo, trace=False)
    return out
